# revision 7
# baseline (speedup 1.0000x reference)
"""GCN encoder (2-layer spmm) on 8 Trainium2 NeuronCores.

Strategy (hardcoded from the sharding hint):
  - Shard dst nodes contiguously across the 8 cores (12500 each, padded to
    12544 = 98 tiles of 128).
  - fc1 (X @ W1 + b1) computed node-sharded on each core, then AllGather the
    bf16 M1 table so every core can gather arbitrary src rows.
  - Edges partitioned by dst owner, grouped by (dst tile, src quarter-group),
    padded to 128-edge chunks.  Per-chunk segment-sum is a matmul with an
    on-device-built weighted one-hot (edge -> local dst) matrix; accumulation
    happens in PSUM across a tile's chunks.
  - fc2 applied per dst tile on the relu'd result (kept transposed in PSUM),
    AllGather M2 (padded to 128 cols for the 256-byte gather-row minimum),
    second spmm identically, f32 output.
  - Gathers use the GPSIMD dma_gather custom instruction (int16 indices ->
    node table split into 4 groups of 25088 rows).
"""

import dataclasses

import numpy as np
import ml_dtypes

from concourse import bass, bacc, tile, mybir, bass_utils

BF16 = ml_dtypes.bfloat16

# Problem constants (must match the grader's setup_inputs()).
N_NODES = 100000
N_EDGES = 1600000
DIN, DH, DO = 256, 128, 64
NCORES = 8
NPC = N_NODES // NCORES          # 12500 true nodes per core
NT = (NPC + 127) // 128          # 98 dst tiles per core
NPC_PAD = NT * 128               # 12544
NTAB = NCORES * NPC_PAD          # 100352 table rows
NGROUPS = 4
GROUP_ROWS = NTAB // NGROUPS     # 25088 (< 32768 so int16 indices work)
ST = 2                           # tiles per gather super-tile (98 = 49 * 2)


def build_program(nt, chg, st, phases="full", rep=1):
    """Build the (identical-per-core) Bass program. nt tiles, chg chunks per
    (tile, group), st tiles per gather call. rep>1 repeats the phase-B
    gather loop (for R-scaled timing of gathersB)."""
    assert nt % st == 0
    n_st = nt // st
    kpt = NGROUPS * chg              # chunks per tile
    slots_pt = kpt * 128             # edge slots per tile
    ntab = NCORES * nt * 128
    group_rows = ntab // NGROUPS

    nc = bacc.Bacc("TRN2", target_bir_lowering=False, debug=False,
                   num_devices=NCORES, num_swdge_queues=4)
    dt = mybir.dt

    def overlap_view(tile_ap, r0, nrows, width):
        """[nrows, width]-shaped view with row stride 128 (overlapping
        windows): descriptor i covers rows i..i+width/128-1."""
        base = tile_ap[r0:r0 + nrows, :]
        return dataclasses.replace(
            base, ap=mybir.VecI64Pair([[128, nrows], [1, width]]))

    xt = nc.dram_tensor("xt", [DIN, nt * 128], dt.bfloat16,
                        kind="ExternalInput").ap()
    w1 = nc.dram_tensor("w1", [DIN, DH], dt.bfloat16, kind="ExternalInput").ap()
    w2 = nc.dram_tensor("w2", [DH, DO], dt.bfloat16, kind="ExternalInput").ap()
    b1 = nc.dram_tensor("b1", [1, DH], dt.float32, kind="ExternalInput").ap()
    b2 = nc.dram_tensor("b2", [1, DO], dt.float32, kind="ExternalInput").ap()
    # per-group packed int16 gather indices: [128, nt*chg*8]
    idxs = [nc.dram_tensor(f"idx{g}", [128, nt * chg * 8], dt.int16,
                           kind="ExternalInput").ap() for g in range(NGROUPS)]
    # per-slot edge weight / local dst, laid out [128, nt*kpt]
    ew = nc.dram_tensor("ew", [128, nt * kpt], dt.bfloat16,
                        kind="ExternalInput").ap()
    edl = nc.dram_tensor("edl", [128, nt * kpt], dt.bfloat16,
                         kind="ExternalInput").ap()
    out = nc.dram_tensor("out", [nt * 128, DO], dt.float32,
                         kind="ExternalOutput").ap()

    with tile.TileContext(nc) as tc:
        with tc.tile_pool(name="dram", bufs=1, space="DRAM") as dram, \
             tc.tile_pool(name="persist", bufs=1) as pp:
            m1_shard = dram.tile([nt * 128, DH], dt.bfloat16)
            m1_full = dram.tile([ntab + 128, DH], dt.bfloat16)
            m2_shard = dram.tile([nt * 128, 128], dt.bfloat16)
            m2_full = dram.tile([ntab + 128, 128], dt.bfloat16)

            # ---- persistent SBUF state ----
            idx_sb = []
            for g in range(NGROUPS):
                t_ = pp.tile([128, nt * chg * 8], dt.int16, name=f"idxsb{g}")
                ncols = nt * chg * 8
                step = 1960
                for c0 in range(0, ncols, step):
                    c1 = min(c0 + step, ncols)
                    nc.sync.dma_start(t_[:, c0:c1], idxs[g][:, c0:c1])
                idx_sb.append(t_)
            ew_sb = pp.tile([128, nt * kpt, 1], dt.bfloat16)
            edl_sb = pp.tile([128, nt * kpt, 1], dt.bfloat16)
            step = 980
            for c0 in range(0, nt * kpt, step):
                c1 = min(c0 + step, nt * kpt)
                nc.sync.dma_start(ew_sb[:, c0:c1, :], ew[:, c0:c1])
                nc.sync.dma_start(edl_sb[:, c0:c1, :], edl[:, c0:c1])
            w2_sb = pp.tile([DH, DO], dt.bfloat16)
            nc.sync.dma_start(w2_sb[:], w2[:])
            b1_sb = pp.tile([128, DH], dt.float32)
            nc.sync.dma_start(b1_sb[:], b1[:].to_broadcast((128, DH)))
            b2_sb = pp.tile([128, DO], dt.float32)
            nc.sync.dma_start(b2_sb[:], b2[:].to_broadcast((128, DO)))
            iota_sb = pp.tile([128, kpt, 128], dt.bfloat16)
            nc.gpsimd.iota(iota_sb[:], [[0, kpt], [1, 128]],
                           channel_multiplier=0,
                           allow_small_or_imprecise_dtypes=True)

            # ---- phase A: M1 = X @ W1 + b1 (node-sharded) ----
            with tc.tile_pool(name="fc1", bufs=1) as fp, \
                 tc.tile_pool(name="fc1p", bufs=2, space="PSUM") as fpp, \
                 tc.tile_pool(name="fc1o", bufs=2) as fpo:
                xt_sb = []
                for k in range(2):
                    t_ = fp.tile([128, nt * 128], dt.bfloat16, name=f"xtsb{k}")
                    ncols = nt * 128
                    step = 1792
                    for c0 in range(0, ncols, step):
                        c1 = min(c0 + step, ncols)
                        nc.sync.dma_start(t_[:, c0:c1],
                                          xt[k * 128:(k + 1) * 128, c0:c1])
                    xt_sb.append(t_)
                w1_sb = fp.tile([128, 2 * DH], dt.bfloat16)
                for k in range(2):
                    nc.sync.dma_start(w1_sb[:, k * DH:(k + 1) * DH],
                                      w1[k * 128:(k + 1) * 128, :])
                for t in range(nt):
                    ps = fpp.tile([128, DH], dt.float32, name="fc1ps")
                    for k in range(2):
                        nc.tensor.matmul(
                            out=ps[:],
                            lhsT=xt_sb[k][:, t * 128:(t + 1) * 128],
                            rhs=w1_sb[:, k * DH:(k + 1) * DH],
                            start=(k == 0), stop=(k == 1))
                    m1_t = fpo.tile([128, DH], dt.bfloat16, name="m1t")
                    nc.vector.tensor_tensor(out=m1_t[:], in0=ps[:],
                                            in1=b1_sb[:],
                                            op=mybir.AluOpType.add)
                    nc.sync.dma_start(m1_shard[t * 128:(t + 1) * 128, :],
                                      m1_t[:])

            nc.gpsimd.collective_compute(
                "AllGather", mybir.AluOpType.bypass,
                replica_groups=[list(range(NCORES))],
                ins=[m1_shard.opt()], outs=[m1_full[0:ntab, :].opt()])

            # ---- phase B: H^T = relu(spmm(M1)); M2 = H @ W2 + b2 ----
            with tc.tile_pool(name="phB", bufs=1) as bp, \
                 tc.tile_pool(name="phBp", bufs=2, space="PSUM") as bpp:
                for s in [x for _ in range(rep) for x in range(n_st)]:
                    gsb = []
                    for g in range(NGROUPS):
                        t_ = bp.tile([128, st * chg, 2 * DH], dt.bfloat16,
                                     name=f"g1_{g}", bufs=2)
                        c0 = s * st * chg * 8
                        nc.gpsimd.dma_gather(
                            out_ap=t_[:],
                            in_ap=overlap_view(m1_full, g * group_rows,
                                               group_rows, 2 * DH),
                            idxs_ap=idx_sb[g][:, c0:c0 + st * chg * 8],
                            num_idxs=st * chg * 128,
                            num_idxs_reg=st * chg * 128,
                            elem_size=2 * DH, elem_step=DH,
                            single_packet=False,
                            queue_num=g)
                        gsb.append(t_)
                    if phases == "gathersB":
                        continue
                    for tl in range(st):
                        t = s * st + tl
                        oh = bp.tile([128, kpt, 128], dt.bfloat16,
                                     name="oh", bufs=2)
                        csl = slice(t * kpt, (t + 1) * kpt)
                        nc.vector.tensor_tensor(
                            out=oh[:],
                            in0=edl_sb[:, csl, :].to_broadcast((128, kpt, 128)),
                            in1=iota_sb[:],
                            op=mybir.AluOpType.is_equal)
                        nc.vector.tensor_tensor(
                            out=oh[:],
                            in0=oh[:],
                            in1=ew_sb[:, csl, :].to_broadcast((128, kpt, 128)),
                            op=mybir.AluOpType.mult)
                        ps_ht = bpp.tile([128, 128], dt.float32, name="psht")
                        for g in range(NGROUPS):
                            for cg in range(chg):
                                k = g * chg + cg
                                nc.tensor.matmul(
                                    out=ps_ht[:],
                                    lhsT=gsb[g][:, tl * chg + cg, 0:DH],
                                    rhs=oh[:, k, :],
                                    start=(k == 0), stop=(k == kpt - 1))
                        ht = bp.tile([128, 128], dt.bfloat16, name="ht", bufs=2)
                        nc.scalar.activation(
                            out=ht[:], in_=ps_ht[:],
                            func=mybir.ActivationFunctionType.Relu)
                        ps_m2 = bpp.tile([128, DO], dt.float32, name="psm2")
                        nc.tensor.matmul(out=ps_m2[:], lhsT=ht[:], rhs=w2_sb[:],
                                         start=True, stop=True)
                        m2_t = bp.tile([128, 128], dt.bfloat16, name="m2t",
                                       bufs=2)
                        nc.vector.tensor_tensor(out=m2_t[:, 0:DO],
                                                in0=ps_m2[:], in1=b2_sb[:],
                                                op=mybir.AluOpType.add)
                        nc.vector.memset(m2_t[:, DO:128], 0)
                        nc.sync.dma_start(m2_shard[t * 128:(t + 1) * 128, :],
                                          m2_t[:])

            if phases == "full":
                nc.gpsimd.collective_compute(
                    "AllGather", mybir.AluOpType.bypass,
                    replica_groups=[list(range(NCORES))],
                    ins=[m2_shard.opt()], outs=[m2_full[0:ntab, :].opt()])

            # ---- phase C: out = spmm(M2) ----
            with tc.tile_pool(name="phC", bufs=1) as cp, \
                 tc.tile_pool(name="phCp", bufs=2, space="PSUM") as cpp:
                for s in (range(n_st) if phases == "full" else []):
                    gsb = []
                    for g in range(NGROUPS):
                        t_ = cp.tile([128, st * chg, 256], dt.bfloat16,
                                     name=f"g2_{g}", bufs=2)
                        c0 = s * st * chg * 8
                        nc.gpsimd.dma_gather(
                            out_ap=t_[:],
                            in_ap=overlap_view(m2_full, g * group_rows,
                                               group_rows, 256),
                            idxs_ap=idx_sb[g][:, c0:c0 + st * chg * 8],
                            num_idxs=st * chg * 128,
                            num_idxs_reg=st * chg * 128,
                            elem_size=256, elem_step=128,
                            single_packet=False,
                            queue_num=g)
                        gsb.append(t_)
                    for tl in range(st):
                        t = s * st + tl
                        oh = cp.tile([128, kpt, 128], dt.bfloat16,
                                     name="ohc", bufs=2)
                        csl = slice(t * kpt, (t + 1) * kpt)
                        nc.vector.tensor_tensor(
                            out=oh[:],
                            in0=edl_sb[:, csl, :].to_broadcast((128, kpt, 128)),
                            in1=iota_sb[:],
                            op=mybir.AluOpType.is_equal)
                        nc.vector.tensor_tensor(
                            out=oh[:],
                            in0=oh[:],
                            in1=ew_sb[:, csl, :].to_broadcast((128, kpt, 128)),
                            op=mybir.AluOpType.mult)
                        ps_o = cpp.tile([128, DO], dt.float32, name="pso")
                        for g in range(NGROUPS):
                            for cg in range(chg):
                                k = g * chg + cg
                                nc.tensor.matmul(
                                    out=ps_o[:],
                                    lhsT=oh[:, k, :],
                                    rhs=gsb[g][:, tl * chg + cg, 0:DO],
                                    start=(k == 0), stop=(k == kpt - 1))
                        o_t = cp.tile([128, DO], dt.float32, name="ot", bufs=2)
                        nc.vector.tensor_copy(out=o_t[:], in_=ps_o[:])
                        nc.sync.dma_start(out[t * 128:(t + 1) * 128, :], o_t[:])

    nc.compile()
    return nc


def prep_inputs(X, edge_src, edge_dst, edge_weight, W1, b1, W2, b2,
                n_nodes, npc, nt, ncores=NCORES):
    """Host-side sharding/packing. Returns (in_maps, chg)."""
    npc_pad = nt * 128
    ntab = ncores * npc_pad
    group_rows = ntab // NGROUPS

    XT = np.ascontiguousarray(X.T).astype(BF16)  # [DIN, n_nodes]

    src_row = ((edge_src // npc) * npc_pad + edge_src % npc).astype(np.int64)
    grp = src_row // group_rows
    dst_core = edge_dst // npc

    # first pass: global max chunk count per (tile, group) cell
    chg = 1
    per_core = []
    for c in range(ncores):
        sel = np.nonzero(dst_core == c)[0]
        dl = edge_dst[sel] - c * npc
        t_ = dl // 128
        cell = t_ * NGROUPS + grp[sel]
        order = np.argsort(cell, kind="stable")
        sel = sel[order]
        cell = cell[order]
        counts = np.bincount(cell, minlength=nt * NGROUPS)
        chg = max(chg, int(np.ceil(counts.max() / 128)))
        per_core.append((sel, cell, counts))

    kpt = NGROUPS * chg
    in_maps = []
    for c in range(ncores):
        sel, cell, counts = per_core[c]
        # position of each edge within its cell
        starts = np.zeros(nt * NGROUPS, np.int64)
        starts[1:] = np.cumsum(counts)[:-1]
        pos = np.arange(len(sel)) - starts[cell]
        slot = cell * (chg * 128) + pos  # slot in [nt * kpt * 128)

        w_flat = np.zeros(nt * kpt * 128, np.float32)
        dl_flat = np.zeros(nt * kpt * 128, np.float32)
        w_flat[slot] = edge_weight[sel]
        dl_flat[slot] = (edge_dst[sel] - c * npc) % 128
        # [128, nt*kpt] with [p, col] = slot col*128+p
        w_arr = w_flat.reshape(nt * kpt, 128).T.astype(BF16).copy()
        dl_arr = dl_flat.reshape(nt * kpt, 128).T.astype(BF16).copy()

        m = {"ew": w_arr, "edl": dl_arr}
        for g in range(NGROUPS):
            flat_g = np.zeros(nt * chg * 128, np.int64)
            eg = grp[sel] == g
            # cell = t*NGROUPS+g -> per-group slot index t*chg*128 + pos
            tg = cell[eg] // NGROUPS
            flat_g[tg * (chg * 128) + pos[eg]] = src_row[sel[eg]] - g * group_rows
            idx16 = flat_g.reshape(-1, 16).T.astype(np.int16)  # [16, nt*chg*8]
            m[f"idx{g}"] = np.ascontiguousarray(np.tile(idx16, (8, 1)))

        xt_c = np.zeros((DIN, npc_pad), BF16)
        xt_c[:, :npc] = XT[:, c * npc:(c + 1) * npc]
        m["xt"] = xt_c
        m["w1"] = W1.astype(BF16)
        m["w2"] = W2.astype(BF16)
        m["b1"] = b1.reshape(1, -1).astype(np.float32)
        m["b2"] = b2.reshape(1, -1).astype(np.float32)
        in_maps.append(m)
    return in_maps, chg


_CACHE = {}


def run(X, edge_src, edge_dst, edge_weight, W1, b1, W2, b2,
        n_nodes, n_edges, npc, nt, st, trace=False):
    in_maps, chg = prep_inputs(X, edge_src, edge_dst, edge_weight, W1, b1,
                               W2, b2, n_nodes, npc, nt)
    key = (nt, chg, st)
    if key not in _CACHE:
        _CACHE[key] = build_program(nt, chg, st)
    nc = _CACHE[key]
    res = bass_utils.run_bass_kernel_spmd(
        nc, in_maps, core_ids=list(range(NCORES)), trace=trace)
    outs = [res.results[c]["out"][:npc] for c in range(NCORES)]
    return np.concatenate(outs, axis=0)[:n_nodes], res


def kernel(X, edge_src, edge_dst, edge_weight, W1, b1, W2, b2):
    X = np.asarray(X, np.float32)
    edge_src = np.asarray(edge_src, np.int32)
    edge_dst = np.asarray(edge_dst, np.int32)
    edge_weight = np.asarray(edge_weight, np.float32)
    out, _ = run(X, edge_src, edge_dst, edge_weight,
                 np.asarray(W1, np.float32), np.asarray(b1, np.float32),
                 np.asarray(W2, np.float32), np.asarray(b2, np.float32),
                 N_NODES, N_EDGES, NPC, NT, ST)
    return out

